# revision 20
# baseline (speedup 1.0000x reference)
"""Trainium2 Bass kernel for nn_LookupFFN (vq_codebook).

reference:  proj = x @ R.T ; idx = argmax(proj, 1) ; out = L[idx]
  x: [16384, 1024] f32, R: [1024, 1024] f32, L: [1024, 1024] f32

Strategy (data-parallel over 8 NeuronCores, 2048 rows of x per core):
  - The argmax needs near-fp32 precision on proj (real top-2 margins go
    down to 6.8e-4 while proj ~ +-130).  Use a 2.0-bf16-pass-equivalent
    mixed-precision split (vs 3 bf16 passes for the baseline):
        proj * 2^11 = (32*xh) @ (64*Rh)          fp16 main term (1x rate)
                    + e4m3(2048*xl) @ e4m3(R)    fp8 DoubleRow (2x rate)
                    + e4m3(x) @ e4m3(2048*Rl)    fp8 DoubleRow (2x rate)
    with xh = fp16(x), xl = x - xh (and same for R).  fp16 keeps 11
    mantissa bits, so the fp8 corrections only carry ~2^-11-scale
    residuals; total proj noise ~1.5e-4 -> ZERO argmax flips on the
    fixed inputs (host-sim rel err 0.0).  All three terms accumulate
    into one PSUM tile at a common 2^11 scale (power-of-two scales are
    exact in fp16; argmax is scale-invariant).  fp8 e4m3 needs the
    2^11 scaling because the residuals (~1e-4) sit below its subnormal
    range.
  - fp8 DoubleRow matmuls contract k=256 per 512-cycle instruction
    (measured 2x bf16 FLOP rate on HW); operands pack k = c*256 +
    two*128 + p as SBUF tiles lhsT [128, 2, 128] / rhs [128, 2, 512].
  - Row-max + argmax via VectorE max/max_index straight from PSUM.
  - out rows fetched exactly (fp32) with a GPSIMD indirect DMA gather of
    L rows by the computed indices.
Perf notes (HW ~142-144us over 8 cores, vs 194us for the 3-pass bf16
baseline; PE floor 2.0 passes = 16 tiles * 16384 cols @2.4GHz ~ 109us,
plus ~8us fixed engine-start preamble, ~5us first-operand DMA, ~6us
PE instruction turnaround, ~13us epilogue tail + end barrier):
  - R chunks stream on the scalar HWDGE queue, rh32 (fp16) first so the
    main terms start early, then r8; rls8 rides the gpsimd queue in
    parallel.  The first matmul only waits on rh chunk 0 + xh pair 0.
  - x tiles load in 2-tile pairs on the sync queue (xh first, fp8
    correction operands after); out stores go on the scalar queue which
    is idle after the R loads.
  - Warm-up (tiles 0-2): fp16 main terms only; their fp8 corrections +
    epilogues are deferred (PEND=3) until the fp8 R chunks land, giving
    PE correction-free work at kernel start.
  - Steady state: main + both corrections + epilogue inline; only the
    final tile's epilogue trails the last matmul.  That tile's gather
    is split into the two 2KB half-rows of L (scaled-index trick) so
    the first half's store overlaps the second half's gather.
"""
import sys

if "/opt/trn_rl_repo" not in sys.path:
    sys.path.insert(0, "/opt/trn_rl_repo")

import ml_dtypes
import numpy as np

import concourse.bass as bass
import concourse.tile as tile
from concourse import bacc, mybir
from concourse.bass import IndirectOffsetOnAxis
from concourse.bass_utils import run_bass_kernel_spmd


def _ensure_axon_hooks_module():
    """Some environments set BASS_TRACE=1; run_bass_kernel_spmd then imports
    antenv.axon_hooks, which this image's antenv package lacks. Provide a
    minimal implementation (ctypes into libaxon_pjrt.so when present)."""
    import contextlib
    import ctypes
    import os
    import types

    if "antenv.axon_hooks" in sys.modules:
        return
    try:
        import antenv
    except ImportError:
        return
    mod = types.ModuleType("antenv.axon_hooks")
    hook_box = [None]
    mod.set_axon_ntff_profile_hook = lambda h: hook_box.__setitem__(0, h)
    mod.get_axon_ntff_profile_hook = lambda: hook_box[0]
    so_path = "/opt/axon/libaxon_pjrt.so"
    if os.path.exists(so_path):
        try:
            lib = ctypes.CDLL(so_path)
            if hasattr(lib, "axon_start_nrt_profile"):
                lib.axon_start_nrt_profile.argtypes = [
                    ctypes.POINTER(ctypes.c_int64),
                    ctypes.c_size_t,
                ]
                lib.axon_start_nrt_profile.restype = ctypes.c_int64
                lib.axon_stop_nrt_profile.argtypes = [ctypes.c_char_p]
                lib.axon_stop_nrt_profile.restype = ctypes.c_int64

                @contextlib.contextmanager
                def _hook(output_dir, device_ids):
                    import jax

                    jax.devices()
                    if device_ids:
                        ids = (ctypes.c_int64 * len(device_ids))(*device_ids)
                        rc = lib.axon_start_nrt_profile(ids, len(device_ids))
                    else:
                        rc = lib.axon_start_nrt_profile(None, 0)
                    if rc != 0:
                        raise RuntimeError(f"axon_start_nrt_profile rc={rc}")
                    try:
                        yield
                    finally:
                        lib.axon_stop_nrt_profile(str(output_dir).encode())

                hook_box[0] = _hook
        except OSError:
            pass
    sys.modules["antenv.axon_hooks"] = mod
    antenv.axon_hooks = mod


_ensure_axon_hooks_module()

F32 = mybir.dt.float32
BF16 = mybir.dt.bfloat16
F16 = mybir.dt.float16
E4 = mybir.dt.float8e4
U32 = mybir.dt.uint32
DR = mybir.MatmulPerfMode.DoubleRow

N = 16384
D = 1024
NB = 1024  # buckets
DOUT = 1024
NCORES = 8
NSHARD = N // NCORES  # 2048 rows per core
KT = D // 128  # 8 k-tiles (bf16)
KP = D // 256  # 4 k-pair tiles (fp8 DoubleRow)
NTILES = NSHARD // 128  # 16 n-tiles per core

_CACHED = {}


def build_nc(n_bufs: int = 5):
    nc = bacc.Bacc("TRN2", target_bir_lowering=False, debug=False)
    xh16 = nc.declare_dram_parameter("xh16", [D, NSHARD], F16, isOutput=False)
    xls8 = nc.declare_dram_parameter("xls8", [D, NSHARD], E4, isOutput=False)
    x8 = nc.declare_dram_parameter("x8", [D, NSHARD], E4, isOutput=False)
    rh32 = nc.declare_dram_parameter("rh32", [D, NB], F16, isOutput=False)
    r8 = nc.declare_dram_parameter("r8", [D, NB], E4, isOutput=False)
    rls8 = nc.declare_dram_parameter("rls8", [D, NB], E4, isOutput=False)
    L = nc.declare_dram_parameter("L", [NB, DOUT], F32, isOutput=False)
    out = nc.declare_dram_parameter("out", [NSHARD, DOUT], F32, isOutput=True)

    with tile.TileContext(nc) as tc:
        with (
            tc.tile_pool(name="rpool", bufs=1) as rpool,
            tc.tile_pool(name="xpool", bufs=n_bufs) as xpool,
            tc.tile_pool(name="gpool", bufs=n_bufs) as gpool,
            tc.tile_pool(name="ipool", bufs=n_bufs) as ipool,
            tc.tile_pool(name="ps", bufs=4, space="PSUM") as ps,
        ):
            # R chunks resident in SBUF; issued on the scalar engine's
            # HWDGE queue so they don't serialize ahead of x-tile loads
            # on the sync queue. Rh32 first: the main terms only need it.
            rh_sb = []  # 8 views [128, NB] fp16, one per k-tile
            r8_sb = []  # 4 views [128, 2, NB] fp8, one per k-pair
            rls_sb = []  # 4 views [128, 2, NB] fp8

            def load_rh_chunk(k2):
                t_ = rpool.tile([128, 2, NB], F16, tag=f"rh{k2}")
                nc.scalar.dma_start(
                    out=t_[:],
                    in_=rh32[k2 * 256 : (k2 + 1) * 256, :].rearrange(
                        "(k p) b -> p k b", k=2
                    ),
                )
                rh_sb.extend([t_[:, 0, :], t_[:, 1, :]])


            def load_r8_chunk(param, dst_list, c, tag, eng=None):
                t_ = rpool.tile([128, 2, NB], E4, tag=tag)
                (eng or nc.scalar).dma_start(
                    out=t_[:],
                    in_=param[c * 256 : (c + 1) * 256, :].rearrange(
                        "(two p) b -> p two b", two=2
                    ),
                )
                dst_list.append(t_[:])

            def load_xh(t):
                # 2-tile (256-col) loads: fewer dma_start issues and
                # larger contiguous bursts per partition row.
                c0 = t * 128
                h = xpool.tile([128, KT, 256], F16, tag="xh")
                nc.sync.dma_start(
                    out=h[:],
                    in_=xh16[:, c0 : c0 + 256].rearrange("(k p) j -> p k j", k=KT),
                )
                return h[:, :, 0:128], h[:, :, 128:256]

            def load_xcorr(t):
                c0 = t * 128
                l8 = xpool.tile([128, KP, 2, 256], E4, tag="xl")
                nc.sync.dma_start(
                    out=l8[:],
                    in_=xls8[:, c0 : c0 + 256].rearrange(
                        "(c two p) j -> p c two j", two=2, p=128
                    ),
                )
                f8 = xpool.tile([128, KP, 2, 256], E4, tag="x8")
                nc.sync.dma_start(
                    out=f8[:],
                    in_=x8[:, c0 : c0 + 256].rearrange(
                        "(c two p) j -> p c two j", two=2, p=128
                    ),
                )
                return (
                    (l8[:, :, :, 0:128], f8[:, :, :, 0:128]),
                    (l8[:, :, :, 128:256], f8[:, :, :, 128:256]),
                )

            def main_term(proj, xh_sb, start):
                for k in range(KT):
                    for bh in range(2):
                        bs = bh * 512
                        nc.tensor.matmul(
                            proj[:, bs : bs + 512],
                            lhsT=xh_sb[:, k, :],
                            rhs=rh_sb[k][:, bs : bs + 512],
                            start=(start and k == 0),
                            stop=False,
                        )

            def corr_terms(proj, xl_sb, x8_sb):
                for x_sb, r_list in ((xl_sb, r8_sb), (x8_sb, rls_sb)):
                    last = r_list is rls_sb
                    for c in range(KP):
                        for bh in range(2):
                            bs = bh * 512
                            nc.tensor.matmul(
                                proj[:, bs : bs + 512],
                                lhsT=x_sb[:, c, :, :],
                                rhs=r_list[c][:, :, bs : bs + 512],
                                start=False,
                                stop=(last and c == KP - 1 and bh == 1),
                                perf_mode=DR,
                            )

            def epilogue(t, proj):
                c0 = t * 128
                max8 = ipool.tile([128, 8], F32, tag="max8")
                idx8 = ipool.tile([128, 8], U32, tag="idx8")
                nc.vector.max(max8[:], proj[:])
                nc.vector.max_index(idx8[:], max8[:], proj[:])
                g_sb = gpool.tile([128, DOUT], F32, tag="g")
                if t == NTILES - 1:
                    # tail-critical: split the gather into the two 2KB
                    # half-rows of L (viewed as [2*NB, DOUT/2], so the
                    # indirect source keeps offset 0) so the first half's
                    # store overlaps the second half's gather.  Half h of
                    # row idx lives at half-row 2*idx + h.
                    Lh = L[:].rearrange("n (h c) -> (n h) c", h=2)
                    idx2 = ipool.tile([128, 2], U32, tag="idx2")
                    nc.vector.tensor_scalar_mul(idx2[:, 0:1], idx8[:, 0:1], 2)
                    nc.vector.tensor_scalar_add(idx2[:, 1:2], idx2[:, 0:1], 1)
                    for h, eng in ((0, nc.scalar), (1, nc.sync)):
                        b0 = h * 512
                        nc.gpsimd.indirect_dma_start(
                            out=g_sb[:, b0 : b0 + 512],
                            out_offset=None,
                            in_=Lh,
                            in_offset=IndirectOffsetOnAxis(
                                ap=idx2[:, h : h + 1], axis=0
                            ),
                        )
                        eng.dma_start(
                            out=out[c0 : c0 + 128, b0 : b0 + 512],
                            in_=g_sb[:, b0 : b0 + 512],
                        )
                    return
                nc.gpsimd.indirect_dma_start(
                    out=g_sb[:],
                    out_offset=None,
                    in_=L[:],
                    in_offset=IndirectOffsetOnAxis(ap=idx8[:, 0:1], axis=0),
                )
                # scalar HWDGE queue: idle after the R loads, keeps the
                # out stores off the x-load (sync) queue.
                nc.scalar.dma_start(out=out[c0 : c0 + 128, :], in_=g_sb[:])

            def finish_tile(t, proj, xl_sb, x8_sb):
                corr_terms(proj, xl_sb, x8_sb)
                epilogue(t, proj)

            def fused_tile(t, xh_sb, xl_sb, x8_sb):
                proj = ps.tile([128, NB], F32, tag="proj")
                main_term(proj, xh_sb, True)
                corr_terms(proj, xl_sb, x8_sb)
                epilogue(t, proj)

            # DMA issue order drives the warm-up critical path: the first
            # matmul needs xh pair 0 (sync queue) + rh chunk 0 (scalar
            # queue) only.  r8 follows rh on the scalar queue; rls8 rides
            # the gpsimd queue so the whole R set lands before the
            # deferred corrections start at PE slot 4.
            xh0a, xh0b = load_xh(0)
            load_rh_chunk(0)
            load_rh_chunk(1)
            xh1a, xh1b = load_xh(2)
            load_rh_chunk(2)
            load_rh_chunk(3)
            for c in range(KP):
                load_r8_chunk(r8, r8_sb, c, f"r8{c}")
            for c in range(KP):
                # gpsimd queue: transfers during the warm-up window with
                # spare HBM bandwidth, off the serialized scalar R stream.
                load_r8_chunk(rls8, rls_sb, c, f"rls{c}", eng=nc.gpsimd)
            c0p = load_xcorr(0)
            c1p = load_xcorr(2)

            # Warm-up: tiles 0-2 run the fp16 main terms only; their fp8
            # corrections + epilogues are deferred (PEND=3) until the fp8
            # R chunks and correction x operands have landed.
            proj0 = ps.tile([128, NB], F32, tag="proj")
            main_term(proj0, xh0a, True)
            proj1 = ps.tile([128, NB], F32, tag="proj")
            main_term(proj1, xh0b, True)
            proj2 = ps.tile([128, NB], F32, tag="proj")
            main_term(proj2, xh1a, True)
            pend = [
                (0, proj0, c0p[0][0], c0p[0][1]),
                (1, proj1, c0p[1][0], c0p[1][1]),
                (2, proj2, c1p[0][0], c1p[0][1]),
            ]

            finish_tile(*pend.pop(0))
            fused_tile(3, xh1b, c1p[1][0], c1p[1][1])
            for tp in range(4, NTILES, 2):
                a, b = load_xh(tp)
                ca, cb = load_xcorr(tp)
                for t, xs, cs in ((tp, a, ca), (tp + 1, b, cb)):
                    if pend:
                        finish_tile(*pend.pop(0))
                    fused_tile(t, xs, cs[0], cs[1])
            while pend:
                finish_tile(*pend.pop(0))
    nc.compile()
    return nc


def _get_nc():
    if "nc" not in _CACHED:
        _CACHED["nc"] = build_nc()
    return _CACHED["nc"]


def _prep_inputs(x, R, L):
    """Host-side split + transpose. Returns per-core input maps."""
    F16NP = np.float16
    E4NP = ml_dtypes.float8_e4m3
    x = np.ascontiguousarray(x, dtype=np.float32)
    R = np.ascontiguousarray(R, dtype=np.float32)
    L = np.ascontiguousarray(L, dtype=np.float32)

    xh = x.astype(F16NP)
    xh32 = xh.astype(np.float32)
    xh16 = (xh32 * 32.0).astype(F16NP)  # exact power-of-two scale in fp16
    xls8 = ((x - xh32) * 2048.0).astype(E4NP)
    x8 = x.astype(E4NP)
    Rh = R.astype(F16NP)
    Rh32f = Rh.astype(np.float32)
    rh32 = (Rh32f * 64.0).astype(F16NP)
    rls8 = ((R - Rh32f) * 2048.0).astype(E4NP)
    r8 = R.astype(E4NP)

    xh16T = np.ascontiguousarray(xh16.T)  # [D, N]
    xls8T = np.ascontiguousarray(xls8.T)
    x8T = np.ascontiguousarray(x8.T)
    rh32T = np.ascontiguousarray(rh32.T)  # [D, NB]
    r8T = np.ascontiguousarray(r8.T)
    rls8T = np.ascontiguousarray(rls8.T)

    in_maps = []
    for c in range(NCORES):
        s = slice(c * NSHARD, (c + 1) * NSHARD)
        in_maps.append(
            {
                "xh16": np.ascontiguousarray(xh16T[:, s]),
                "xls8": np.ascontiguousarray(xls8T[:, s]),
                "x8": np.ascontiguousarray(x8T[:, s]),
                "rh32": rh32T,
                "r8": r8T,
                "rls8": rls8T,
                "L": L,
            }
        )
    return in_maps


def run(x, R, L, trace=False, **kw):
    nc = _get_nc()
    in_maps = _prep_inputs(x, R, L)
    res = run_bass_kernel_spmd(
        nc, in_maps, core_ids=list(range(NCORES)), trace=trace, **kw
    )
    out = np.concatenate([res.results[c]["out"] for c in range(NCORES)], axis=0)
    return out, res


def kernel(x, R, L):
    out, _ = run(x, R, L, trace=False)
    return out


if __name__ == "__main__":
    rng = np.random.default_rng(0)
    x = rng.standard_normal((N, D), dtype=np.float32)
    R = rng.standard_normal((NB, D), dtype=np.float32)
    L = rng.standard_normal((NB, DOUT), dtype=np.float32)
    out = kernel(x, R, L)
    proj = x.astype(np.float64) @ R.astype(np.float64).T
    idx = np.argmax(proj, axis=1)
    exp = L[idx]
    bad = (out != exp).any(axis=1).sum()
    print("rows mismatching exact-gather expectation:", int(bad))


# revision 21
# speedup vs baseline: 1.0285x; 1.0285x over previous
"""Trainium2 Bass kernel for nn_LookupFFN (vq_codebook).

reference:  proj = x @ R.T ; idx = argmax(proj, 1) ; out = L[idx]
  x: [16384, 1024] f32, R: [1024, 1024] f32, L: [1024, 1024] f32

Strategy (data-parallel over 8 NeuronCores, 2048 rows of x per core):
  - The argmax needs near-fp32 precision on proj (real top-2 margins go
    down to 6.8e-4 while proj ~ +-130).  Use a 2.0-bf16-pass-equivalent
    mixed-precision split (vs 3 bf16 passes for the baseline):
        proj * 2^11 = (32*xh) @ (64*Rh)          fp16 main term (1x rate)
                    + e4m3(2048*xl) @ e4m3(R)    fp8 DoubleRow (2x rate)
                    + e4m3(x) @ e4m3(2048*Rl)    fp8 DoubleRow (2x rate)
    with xh = fp16(x), xl = x - xh (and same for R).  fp16 keeps 11
    mantissa bits, so the fp8 corrections only carry ~2^-11-scale
    residuals; total proj noise ~1.5e-4 -> ZERO argmax flips on the
    fixed inputs (host-sim rel err 0.0).  All three terms accumulate
    into one PSUM tile at a common 2^11 scale (power-of-two scales are
    exact in fp16; argmax is scale-invariant).  fp8 e4m3 needs the
    2^11 scaling because the residuals (~1e-4) sit below its subnormal
    range.
  - fp8 DoubleRow matmuls contract k=256 per 512-cycle instruction
    (measured 2x bf16 FLOP rate on HW); operands pack k = c*256 +
    two*128 + p as SBUF tiles lhsT [128, 2, 128] / rhs [128, 2, 512].
  - Row-max + argmax via VectorE max/max_index straight from PSUM.
  - out rows fetched exactly (fp32) with a GPSIMD indirect DMA gather of
    L rows by the computed indices.
Perf notes (HW ~142-144us over 8 cores, vs 194us for the 3-pass bf16
baseline; PE floor 2.0 passes = 16 tiles * 16384 cols @2.4GHz ~ 109us,
plus ~8us fixed engine-start preamble, ~5us first-operand DMA, ~6us
PE instruction turnaround, ~13us epilogue tail + end barrier):
  - R chunks stream on the scalar HWDGE queue, rh32 (fp16) first so the
    main terms start early, then r8; rls8 rides the gpsimd queue in
    parallel.  The first matmul only waits on rh chunk 0 + xh pair 0.
  - x tiles load in 2-tile pairs on the sync queue (xh first, fp8
    correction operands after); out stores go on the scalar queue which
    is idle after the R loads.
  - Warm-up (tiles 0-2): fp16 main terms only; their fp8 corrections +
    epilogues are deferred (PEND=3) until the fp8 R chunks land, giving
    PE correction-free work at kernel start.
  - Steady state: main + both corrections + epilogue inline; only the
    final tile's epilogue trails the last matmul.  That tile's gather
    is split into the two 2KB half-rows of L (scaled-index trick) so
    the first half's store overlaps the second half's gather.
"""
import sys

if "/opt/trn_rl_repo" not in sys.path:
    sys.path.insert(0, "/opt/trn_rl_repo")

import ml_dtypes
import numpy as np

import concourse.bass as bass
import concourse.tile as tile
from concourse import bacc, mybir
from concourse.bass import IndirectOffsetOnAxis
from concourse.bass_utils import run_bass_kernel_spmd


def _ensure_axon_hooks_module():
    """Some environments set BASS_TRACE=1; run_bass_kernel_spmd then imports
    antenv.axon_hooks, which this image's antenv package lacks. Provide a
    minimal implementation (ctypes into libaxon_pjrt.so when present)."""
    import contextlib
    import ctypes
    import os
    import types

    if "antenv.axon_hooks" in sys.modules:
        return
    try:
        import antenv
    except ImportError:
        return
    mod = types.ModuleType("antenv.axon_hooks")
    hook_box = [None]
    mod.set_axon_ntff_profile_hook = lambda h: hook_box.__setitem__(0, h)
    mod.get_axon_ntff_profile_hook = lambda: hook_box[0]
    so_path = "/opt/axon/libaxon_pjrt.so"
    if os.path.exists(so_path):
        try:
            lib = ctypes.CDLL(so_path)
            if hasattr(lib, "axon_start_nrt_profile"):
                lib.axon_start_nrt_profile.argtypes = [
                    ctypes.POINTER(ctypes.c_int64),
                    ctypes.c_size_t,
                ]
                lib.axon_start_nrt_profile.restype = ctypes.c_int64
                lib.axon_stop_nrt_profile.argtypes = [ctypes.c_char_p]
                lib.axon_stop_nrt_profile.restype = ctypes.c_int64

                @contextlib.contextmanager
                def _hook(output_dir, device_ids):
                    import jax

                    jax.devices()
                    if device_ids:
                        ids = (ctypes.c_int64 * len(device_ids))(*device_ids)
                        rc = lib.axon_start_nrt_profile(ids, len(device_ids))
                    else:
                        rc = lib.axon_start_nrt_profile(None, 0)
                    if rc != 0:
                        raise RuntimeError(f"axon_start_nrt_profile rc={rc}")
                    try:
                        yield
                    finally:
                        lib.axon_stop_nrt_profile(str(output_dir).encode())

                hook_box[0] = _hook
        except OSError:
            pass
    sys.modules["antenv.axon_hooks"] = mod
    antenv.axon_hooks = mod


_ensure_axon_hooks_module()

F32 = mybir.dt.float32
BF16 = mybir.dt.bfloat16
F16 = mybir.dt.float16
E4 = mybir.dt.float8e4
U32 = mybir.dt.uint32
DR = mybir.MatmulPerfMode.DoubleRow

N = 16384
D = 1024
NB = 1024  # buckets
DOUT = 1024
NCORES = 8
NSHARD = N // NCORES  # 2048 rows per core
KT = D // 128  # 8 k-tiles (bf16)
KP = D // 256  # 4 k-pair tiles (fp8 DoubleRow)
NTILES = NSHARD // 128  # 16 n-tiles per core

_CACHED = {}


def build_nc(n_bufs: int = 5):
    nc = bacc.Bacc("TRN2", target_bir_lowering=False, debug=False)
    xh16 = nc.declare_dram_parameter("xh16", [D, NSHARD], F16, isOutput=False)
    xls8 = nc.declare_dram_parameter("xls8", [D, NSHARD], E4, isOutput=False)
    x8 = nc.declare_dram_parameter("x8", [D, NSHARD], E4, isOutput=False)
    rh32 = nc.declare_dram_parameter("rh32", [D, NB], F16, isOutput=False)
    r8 = nc.declare_dram_parameter("r8", [D, NB], E4, isOutput=False)
    rls8 = nc.declare_dram_parameter("rls8", [D, NB], E4, isOutput=False)
    L = nc.declare_dram_parameter("L", [NB, DOUT], F32, isOutput=False)
    out = nc.declare_dram_parameter("out", [NSHARD, DOUT], F32, isOutput=True)

    with tile.TileContext(nc) as tc:
        with (
            tc.tile_pool(name="rpool", bufs=1) as rpool,
            tc.tile_pool(name="xpool", bufs=n_bufs) as xpool,
            tc.tile_pool(name="gpool", bufs=n_bufs) as gpool,
            tc.tile_pool(name="ipool", bufs=n_bufs) as ipool,
            tc.tile_pool(name="ps", bufs=4, space="PSUM") as ps,
        ):
            # R chunks resident in SBUF; issued on the scalar engine's
            # HWDGE queue so they don't serialize ahead of x-tile loads
            # on the sync queue. Rh32 first: the main terms only need it.
            rh_sb = []  # 8 views [128, NB] fp16, one per k-tile
            r8_sb = []  # 4 views [128, 2, NB] fp8, one per k-pair
            rls_sb = []  # 4 views [128, 2, NB] fp8

            def load_rh_chunk(k2, eng=None):
                t_ = rpool.tile([128, 2, NB], F16, tag=f"rh{k2}")
                (eng or nc.scalar).dma_start(
                    out=t_[:],
                    in_=rh32[k2 * 256 : (k2 + 1) * 256, :].rearrange(
                        "(k p) b -> p k b", k=2
                    ),
                )
                rh_sb.extend([t_[:, 0, :], t_[:, 1, :]])


            def load_r8_chunk(param, dst_list, c, tag, eng=None):
                t_ = rpool.tile([128, 2, NB], E4, tag=tag)
                (eng or nc.scalar).dma_start(
                    out=t_[:],
                    in_=param[c * 256 : (c + 1) * 256, :].rearrange(
                        "(two p) b -> p two b", two=2
                    ),
                )
                dst_list.append(t_[:])

            def load_xh(t):
                # 2-tile (256-col) loads: fewer dma_start issues and
                # larger contiguous bursts per partition row.
                c0 = t * 128
                h = xpool.tile([128, KT, 256], F16, tag="xh")
                nc.sync.dma_start(
                    out=h[:],
                    in_=xh16[:, c0 : c0 + 256].rearrange("(k p) j -> p k j", k=KT),
                )
                return h[:, :, 0:128], h[:, :, 128:256]

            def load_xcorr(t):
                c0 = t * 128
                l8 = xpool.tile([128, KP, 2, 256], E4, tag="xl")
                nc.sync.dma_start(
                    out=l8[:],
                    in_=xls8[:, c0 : c0 + 256].rearrange(
                        "(c two p) j -> p c two j", two=2, p=128
                    ),
                )
                f8 = xpool.tile([128, KP, 2, 256], E4, tag="x8")
                nc.sync.dma_start(
                    out=f8[:],
                    in_=x8[:, c0 : c0 + 256].rearrange(
                        "(c two p) j -> p c two j", two=2, p=128
                    ),
                )
                return (
                    (l8[:, :, :, 0:128], f8[:, :, :, 0:128]),
                    (l8[:, :, :, 128:256], f8[:, :, :, 128:256]),
                )

            def main_term(proj, xh_sb, start):
                for k in range(KT):
                    for bh in range(2):
                        bs = bh * 512
                        nc.tensor.matmul(
                            proj[:, bs : bs + 512],
                            lhsT=xh_sb[:, k, :],
                            rhs=rh_sb[k][:, bs : bs + 512],
                            start=(start and k == 0),
                            stop=False,
                        )

            def corr_terms(proj, xl_sb, x8_sb):
                for x_sb, r_list in ((xl_sb, r8_sb), (x8_sb, rls_sb)):
                    last = r_list is rls_sb
                    for c in range(KP):
                        for bh in range(2):
                            bs = bh * 512
                            nc.tensor.matmul(
                                proj[:, bs : bs + 512],
                                lhsT=x_sb[:, c, :, :],
                                rhs=r_list[c][:, :, bs : bs + 512],
                                start=False,
                                stop=(last and c == KP - 1 and bh == 1),
                                perf_mode=DR,
                            )

            def epilogue(t, proj):
                c0 = t * 128
                max8 = ipool.tile([128, 8], F32, tag="max8")
                idx8 = ipool.tile([128, 8], U32, tag="idx8")
                nc.vector.max(max8[:], proj[:])
                nc.vector.max_index(idx8[:], max8[:], proj[:])
                g_sb = gpool.tile([128, DOUT], F32, tag="g")
                if t == NTILES - 1:
                    # tail-critical: split the gather into the two 2KB
                    # half-rows of L (viewed as [2*NB, DOUT/2], so the
                    # indirect source keeps offset 0) so the first half's
                    # store overlaps the second half's gather.  Half h of
                    # row idx lives at half-row 2*idx + h.
                    Lh = L[:].rearrange("n (h c) -> (n h) c", h=2)
                    idx2 = ipool.tile([128, 2], U32, tag="idx2")
                    nc.vector.tensor_scalar_mul(idx2[:, 0:1], idx8[:, 0:1], 2)
                    nc.vector.tensor_scalar_add(idx2[:, 1:2], idx2[:, 0:1], 1)
                    for h, eng in ((0, nc.scalar), (1, nc.sync)):
                        b0 = h * 512
                        nc.gpsimd.indirect_dma_start(
                            out=g_sb[:, b0 : b0 + 512],
                            out_offset=None,
                            in_=Lh,
                            in_offset=IndirectOffsetOnAxis(
                                ap=idx2[:, h : h + 1], axis=0
                            ),
                        )
                        eng.dma_start(
                            out=out[c0 : c0 + 128, b0 : b0 + 512],
                            in_=g_sb[:, b0 : b0 + 512],
                        )
                    return
                nc.gpsimd.indirect_dma_start(
                    out=g_sb[:],
                    out_offset=None,
                    in_=L[:],
                    in_offset=IndirectOffsetOnAxis(ap=idx8[:, 0:1], axis=0),
                )
                # scalar HWDGE queue: idle after the R loads, keeps the
                # out stores off the x-load (sync) queue.
                nc.scalar.dma_start(out=out[c0 : c0 + 128, :], in_=g_sb[:])

            def finish_tile(t, proj, xl_sb, x8_sb):
                corr_terms(proj, xl_sb, x8_sb)
                epilogue(t, proj)

            def fused_tile(t, xh_sb, xl_sb, x8_sb):
                proj = ps.tile([128, NB], F32, tag="proj")
                main_term(proj, xh_sb, True)
                corr_terms(proj, xl_sb, x8_sb)
                epilogue(t, proj)

            # DMA issue order drives the warm-up critical path: the first
            # matmul needs xh pair 0 (sync queue) + rh chunk 0 (scalar
            # queue) only.  r8 follows rh on the scalar queue; rls8 rides
            # the gpsimd queue so the whole R set lands before the
            # deferred corrections start at PE slot 4.
            xh0a, xh0b = load_xh(0)
            load_rh_chunk(0)
            load_rh_chunk(1)
            # k4-k5 ride first on the gpsimd queue: they land ~10us, well
            # before tile 0's k-loop reaches them (~16us) -- the scalar
            # queue alone delivered them ~2us late (PE gap at k=4).
            load_rh_chunk(2, eng=nc.gpsimd)
            xh1a, xh1b = load_xh(2)
            load_rh_chunk(3)
            for c in range(KP):
                load_r8_chunk(r8, r8_sb, c, f"r8{c}")
            for c in range(KP):
                # gpsimd queue: transfers during the warm-up window with
                # spare HBM bandwidth, off the serialized scalar R stream.
                load_r8_chunk(rls8, rls_sb, c, f"rls{c}", eng=nc.gpsimd)
            c0p = load_xcorr(0)
            c1p = load_xcorr(2)

            # Warm-up: tiles 0-2 run the fp16 main terms only; their fp8
            # corrections + epilogues are deferred (PEND=3) until the fp8
            # R chunks and correction x operands have landed.
            proj0 = ps.tile([128, NB], F32, tag="proj")
            main_term(proj0, xh0a, True)
            proj1 = ps.tile([128, NB], F32, tag="proj")
            main_term(proj1, xh0b, True)
            proj2 = ps.tile([128, NB], F32, tag="proj")
            main_term(proj2, xh1a, True)
            pend = [
                (0, proj0, c0p[0][0], c0p[0][1]),
                (1, proj1, c0p[1][0], c0p[1][1]),
                (2, proj2, c1p[0][0], c1p[0][1]),
            ]

            finish_tile(*pend.pop(0))
            fused_tile(3, xh1b, c1p[1][0], c1p[1][1])
            for tp in range(4, NTILES, 2):
                a, b = load_xh(tp)
                ca, cb = load_xcorr(tp)
                for t, xs, cs in ((tp, a, ca), (tp + 1, b, cb)):
                    if pend:
                        finish_tile(*pend.pop(0))
                    fused_tile(t, xs, cs[0], cs[1])
            while pend:
                finish_tile(*pend.pop(0))
    nc.compile()
    return nc


def _get_nc():
    if "nc" not in _CACHED:
        _CACHED["nc"] = build_nc()
    return _CACHED["nc"]


def _prep_inputs(x, R, L):
    """Host-side split + transpose. Returns per-core input maps."""
    F16NP = np.float16
    E4NP = ml_dtypes.float8_e4m3
    x = np.ascontiguousarray(x, dtype=np.float32)
    R = np.ascontiguousarray(R, dtype=np.float32)
    L = np.ascontiguousarray(L, dtype=np.float32)

    xh = x.astype(F16NP)
    xh32 = xh.astype(np.float32)
    xh16 = (xh32 * 32.0).astype(F16NP)  # exact power-of-two scale in fp16
    xls8 = ((x - xh32) * 2048.0).astype(E4NP)
    x8 = x.astype(E4NP)
    Rh = R.astype(F16NP)
    Rh32f = Rh.astype(np.float32)
    rh32 = (Rh32f * 64.0).astype(F16NP)
    rls8 = ((R - Rh32f) * 2048.0).astype(E4NP)
    r8 = R.astype(E4NP)

    xh16T = np.ascontiguousarray(xh16.T)  # [D, N]
    xls8T = np.ascontiguousarray(xls8.T)
    x8T = np.ascontiguousarray(x8.T)
    rh32T = np.ascontiguousarray(rh32.T)  # [D, NB]
    r8T = np.ascontiguousarray(r8.T)
    rls8T = np.ascontiguousarray(rls8.T)

    in_maps = []
    for c in range(NCORES):
        s = slice(c * NSHARD, (c + 1) * NSHARD)
        in_maps.append(
            {
                "xh16": np.ascontiguousarray(xh16T[:, s]),
                "xls8": np.ascontiguousarray(xls8T[:, s]),
                "x8": np.ascontiguousarray(x8T[:, s]),
                "rh32": rh32T,
                "r8": r8T,
                "rls8": rls8T,
                "L": L,
            }
        )
    return in_maps


def run(x, R, L, trace=False, **kw):
    nc = _get_nc()
    in_maps = _prep_inputs(x, R, L)
    res = run_bass_kernel_spmd(
        nc, in_maps, core_ids=list(range(NCORES)), trace=trace, **kw
    )
    out = np.concatenate([res.results[c]["out"] for c in range(NCORES)], axis=0)
    return out, res


def kernel(x, R, L):
    out, _ = run(x, R, L, trace=False)
    return out


if __name__ == "__main__":
    rng = np.random.default_rng(0)
    x = rng.standard_normal((N, D), dtype=np.float32)
    R = rng.standard_normal((NB, D), dtype=np.float32)
    L = rng.standard_normal((NB, DOUT), dtype=np.float32)
    out = kernel(x, R, L)
    proj = x.astype(np.float64) @ R.astype(np.float64).T
    idx = np.argmax(proj, axis=1)
    exp = L[idx]
    bad = (out != exp).any(axis=1).sum()
    print("rows mismatching exact-gather expectation:", int(bad))
